# revision 19
# baseline (speedup 1.0000x reference)
"""Expert-parallel fp8(e4m3fn) dequant kernel for Trainium2 (8 NeuronCores).

Problem: weight (64, 4096, 1024) uint8 = raw fp8_e4m3fn bit patterns,
scale (64, 4096) fp32. Output (64, 4096, 1024) bf16 =
fp8_decode(weight) * bf16(scale)[..., None].

TRN2's native FP8_EXP4 is NOT OCP e4m3fn (exponent=1111 decodes to
Inf/NaN instead of 256..448), so we decode with integer/fp16 bit
arithmetic instead of the hardware fp8 path:

    B(u16) = x*128 + (x>=128)*16384        # = sign<<15 | mag<<7
    view B as fp16  ->  value = w * 2^-8   (exact, incl. subnormals)
    out_bf16 = fp16(B) * (bf16(scale) * 256)   # exact product, single RNE

which is bitwise identical to jax's  fp8.astype(bf16) * scale.astype(bf16).

Sharding: dim 0 (experts) split 8 ways; zero communication.
Engine split per super-tile [128 partitions x 8192 bytes]:
  ACT: t1 = x*128             (u8 -> u16)
  DVE: t2 = (x>=128)*16384 ; t1 += t2 ; out = fp16(t1) * scale_row -> bf16
"""
import sys

if '/opt/trn_rl_repo' not in sys.path:
    sys.path.insert(0, '/opt/trn_rl_repo')

import numpy as np
import ml_dtypes

E, O, I = 64, 4096, 1024
N_CORES = 8
E_PER = E // N_CORES          # 8 experts per core
R = E_PER * O                 # 32768 rows per core
P = 128                       # partitions
RPP = 8                       # rows per partition per super-tile
G = R // (P * RPP)            # 32 super-tiles per core
FD = RPP * I                  # 8192 bytes free-dim per super-tile

_cache = {}


def _build_nc(repeat=1, stages=("act", "dve2", "add", "mul"), bufs=3):
    import concourse.bacc as bacc
    import concourse.mybir as mybir
    from concourse.mybir import AluOpType as A
    from concourse.tile import TileContext

    u8, u16, f32 = mybir.dt.uint8, mybir.dt.uint16, mybir.dt.float32
    bf16, fp16 = mybir.dt.bfloat16, mybir.dt.float16

    nc = bacc.Bacc(trn_type="TRN2", enable_partition_id=False)
    w = nc.dram_tensor("w", [R, I], u8, kind="ExternalInput")
    s = nc.dram_tensor("s", [P, G * RPP], f32, kind="ExternalInput")
    y = nc.dram_tensor("y", [R, I], bf16, kind="ExternalOutput")

    # row r = (g*128 + p)*RPP + j  ->  [g, p, (j i)]
    wv = w.rearrange("(g p j) i -> g p (j i)", p=P, j=RPP)
    yv = y.rearrange("(g p j) i -> g p (j i)", p=P, j=RPP)

    with TileContext(nc) as tc:
        with tc.tile_pool(name="scp", bufs=1) as scp, \
             tc.tile_pool(name="pool", bufs=bufs) as pool:
            # scale prep: round to bf16 (RNE) then *256, all on DVE
            st0 = scp.tile([P, G * RPP], f32)
            nc.sync.dma_start(st0[:], s[:])
            sbf = scp.tile([P, G * RPP], bf16)
            nc.vector.tensor_copy(sbf[:], st0[:])
            s2 = scp.tile([P, G * RPP], f32)
            nc.vector.tensor_scalar(s2[:], sbf[:], 256.0, None, A.mult)
            # ACT's own copy of the scale vector (pre-touched on ACT so later
            # activation ops carry only same-engine deps on it)
            s2a = scp.tile([P, G * RPP], f32)
            nc.scalar.copy(s2a[:], s2[:])

            for g in [g for _ in range(repeat) for g in range(G)]:
                xt = pool.tile([P, FD], u8, tag="xt")
                nc.sync.dma_start(xt[:], wv[g])
                t1 = pool.tile([P, FD], u16, tag="t1")
                if "act" in stages:
                    nc.scalar.mul(t1[:], xt[:], 128.0)
                elif "dve1" in stages:
                    nc.vector.tensor_scalar(t1[:], xt[:], 128.0, None, A.mult)
                t2 = pool.tile([P, FD], u16, tag="t2")
                if "dve2b" in stages:
                    # 16-bit source: t2 = (t1 >= 16384)*16384, eligible for 4x mode
                    nc.vector.tensor_scalar(t2[:], t1[:], 16384.0, 16384.0, A.is_ge, A.mult)
                elif "gp" in stages:
                    nc.gpsimd.tensor_scalar(t2[:], xt[:], 128.0, 16384.0, A.is_ge, A.mult)
                elif "dve2" in stages:
                    nc.vector.tensor_scalar(t2[:], xt[:], 128.0, 16384.0, A.is_ge, A.mult)
                if "adds" in stages:
                    # sliced add: FD-256 chunks stream under the DVE drain threshold
                    CH = 256
                    for a0 in range(0, FD, CH):
                        nc.vector.tensor_tensor(t1[:, a0:a0 + CH], t1[:, a0:a0 + CH],
                                                t2[:, a0:a0 + CH], A.add)
                elif "add" in stages:
                    nc.vector.tensor_tensor(t1[:], t1[:], t2[:], A.add)
                ot = pool.tile([P, FD], bf16, tag="ot")
                if "mul" in stages:
                    for j in range(RPP):
                        nc.vector.tensor_scalar(
                            ot[:, j * I:(j + 1) * I],
                            t1[:, j * I:(j + 1) * I].bitcast(fp16),
                            s2[:, g * RPP + j: g * RPP + j + 1], None, A.mult)
                elif "mulact" in stages:
                    import concourse.mybir as _mb
                    for j in range(RPP):
                        nc.scalar.activation(
                            ot[:, j * I:(j + 1) * I],
                            t1[:, j * I:(j + 1) * I].bitcast(fp16),
                            _mb.ActivationFunctionType.Copy,
                            scale=s2[:, g * RPP + j: g * RPP + j + 1])
                elif any(s.startswith("mulg") for s in stages):
                    # split at supertile granularity: every Nth supertile's muls on ACT
                    import concourse.mybir as _mb
                    frac = next(s for s in stages if s.startswith("mulg"))
                    act_mod = int(frac[4:] or 4)
                    on_act = (g % act_mod == act_mod - 1)
                    for j in range(RPP):
                        o = ot[:, j * I:(j + 1) * I]
                        t = t1[:, j * I:(j + 1) * I].bitcast(fp16)
                        sa = s2[:, g * RPP + j: g * RPP + j + 1]
                        saa = s2a[:, g * RPP + j: g * RPP + j + 1]
                        if on_act:
                            nc.scalar.activation(o, t, _mb.ActivationFunctionType.Copy, scale=saa)
                        else:
                            nc.vector.tensor_scalar(o, t, sa, None, A.mult)
                elif any(s.startswith("muls") for s in stages):
                    # split per-row scale ops: every Nth row goes to ACT
                    import concourse.mybir as _mb
                    frac = next(s for s in stages if s.startswith("muls"))
                    act_mod = int(frac[4:] or 3)  # 1 of act_mod rows on ACT
                    for j in range(RPP):
                        o = ot[:, j * I:(j + 1) * I]
                        t = t1[:, j * I:(j + 1) * I].bitcast(fp16)
                        sa = s2[:, g * RPP + j: g * RPP + j + 1]
                        saa = s2a[:, g * RPP + j: g * RPP + j + 1]
                        if j % act_mod == act_mod - 1:
                            nc.scalar.activation(o, t, _mb.ActivationFunctionType.Copy, scale=saa)
                        else:
                            nc.vector.tensor_scalar(o, t, sa, None, A.mult)
                elif "mulimm" in stages:
                    # timing probe: per-row ops with immediate scalar (wrong values)
                    for j in range(RPP):
                        nc.vector.tensor_scalar(
                            ot[:, j * I:(j + 1) * I],
                            t1[:, j * I:(j + 1) * I].bitcast(fp16),
                            3.14, None, A.mult)
                elif "mulone" in stages:
                    # timing probe: one whole-supertile TS with immediate scalar
                    nc.vector.tensor_scalar(
                        ot[:], t1[:].bitcast(fp16), 3.14, None, A.mult)
                elif "mulbc" in stages:
                    t3f = t1[:].bitcast(fp16).rearrange("p (j i) -> p j i", j=RPP)
                    o3 = ot[:].rearrange("p (j i) -> p j i", j=RPP)
                    scb = s2[:, g * RPP:(g + 1) * RPP].unsqueeze(2).to_broadcast((P, RPP, I))
                    nc.vector.tensor_tensor(o3, t3f, scb, A.mult)
                elif "mulcopy" in stages:
                    nc.vector.tensor_copy(ot[:], t1[:].bitcast(bf16))
                if not any(s in stages for s in ("mul", "mulact", "mulcopy")):
                    # DMA-traffic-only variant: ship xt's bytes out twice
                    xb = xt[:].bitcast(bf16)
                    nc.sync.dma_start(yv[g][:, :FD // 2], xb)
                    nc.sync.dma_start(yv[g][:, FD // 2:], xb)
                else:
                    nc.sync.dma_start(yv[g], ot[:])

    nc.compile()
    return nc


def _prep_scale(scale_c: np.ndarray) -> np.ndarray:
    """scale_c: (R,) fp32 for this core -> [P, G*RPP] with
    sp[p, g*RPP+j] = scale_c[(g*128+p)*RPP + j]."""
    return np.ascontiguousarray(
        scale_c.reshape(G, P, RPP).transpose(1, 0, 2).reshape(P, G * RPP))


def kernel(weight: np.ndarray, scale: np.ndarray) -> np.ndarray:
    from concourse import bass_utils

    weight = np.asarray(weight)
    scale = np.asarray(scale)
    assert weight.shape == (E, O, I) and scale.shape == (E, O)
    w8 = weight.view(np.uint8) if weight.dtype != np.uint8 else weight
    sc = scale.astype(np.float32, copy=False)

    import os
    stages = tuple(os.environ.get("K_STAGES", "act+dve2+add+mul").split("+"))
    if _cache.get("stages") != stages:
        _cache["nc"] = _build_nc(stages=stages)
        _cache["stages"] = stages
    nc = _cache["nc"]

    in_maps = []
    for c in range(N_CORES):
        wc = np.ascontiguousarray(w8[c * E_PER:(c + 1) * E_PER].reshape(R, I))
        scc = _prep_scale(np.ascontiguousarray(
            sc[c * E_PER:(c + 1) * E_PER].reshape(R)))
        in_maps.append({"w": wc, "s": scc})

    res = bass_utils.run_bass_kernel_spmd(nc, in_maps, core_ids=list(range(N_CORES)))
    out = np.empty((E, O, I), dtype=ml_dtypes.bfloat16)
    for c in range(N_CORES):
        yc = np.asarray(res.results[c]["y"]).reshape(E_PER, O, I)
        out[c * E_PER:(c + 1) * E_PER] = yc
    return out
